# revision 2
# baseline (speedup 1.0000x reference)
"""GQA attention core (B=2,S=2048,HQ=32,HKV=8,D=64) + out-proj on 8 NeuronCores.

Sharding: pure data/sequence parallel. Core c handles batch b=c//4 and Q-row
chunk qc=c%4 (512 rows). Each core holds the full K/V of its batch plus the
whole (replicated) W_out, computes its 512 output rows completely, no
collectives. Host pre-transposes operands into the d-major layouts the PE
array needs so no on-chip transposes are required:

  scores^T[k,q] = kT[d,k].T @ qT[d,q]   (per q-head; kT/qT prepped on host)
  softmax along partition dim k, no max-subtraction (scores ~ N(0,1)),
  sums via a ones-column appended to V:  oT'[65,q] = vE[k,65].T @ exp(sT)
  normalize rows 0..63 by row 64, out[q,:] = sum_t oT[128t:,q].T @ W^T[128t:,:]

All matmuls in bf16 (inputs pre-cast on host), accumulation fp32 in PSUM.
"""

import math
from contextlib import ExitStack

import numpy as np
import ml_dtypes

import concourse.bass as bass
import concourse.bacc as bacc
import concourse.tile as tile
from concourse import mybir
from concourse.bass_utils import run_bass_kernel_spmd

BF16 = ml_dtypes.bfloat16

B, S, HQ, HKV, D, HID = 2, 2048, 32, 8, 64, 2048
GRP = HQ // HKV          # 4 q-heads per kv head
NC_PER_B = 4             # q-chunks per batch
SQ = S // NC_PER_B       # 512 q rows per core
SK = S
KT = SK // 128           # 16 k tiles
VE = 66                  # dv(64) + ones col + pad for 4B alignment
HD = HQ * D              # 2048 concat head dim
PROJ_T = HD // 128       # 16
HID_T = HID // 512       # 4
QT_N = (HKV // 2) * GRP  # 16 qT slots
SCALE = 1.0 / math.sqrt(D)

FP32 = mybir.dt.float32
BF = mybir.dt.bfloat16

_cached = None


def _build_program():
    nc = bacc.Bacc("TRN2", target_bir_lowering=False, debug=False)
    qT_d = nc.dram_tensor("qT", [128, QT_N, SQ], BF, kind="ExternalInput")
    kT_d = nc.dram_tensor("kT", [128, HKV // 2, SK], BF, kind="ExternalInput")
    vE_d = nc.dram_tensor("vE", [128, HKV, KT, VE], BF, kind="ExternalInput")
    wT_d = nc.dram_tensor("wT", [128, PROJ_T, HID], BF, kind="ExternalInput")
    out_d = nc.dram_tensor("out", [SQ, HID], FP32, kind="ExternalOutput")

    with ExitStack() as ctx:
        tc = ctx.enter_context(tile.TileContext(nc))
        singles = ctx.enter_context(tc.tile_pool(name="singles", bufs=1))
        qk_pool = ctx.enter_context(tc.tile_pool(name="qk", bufs=3, space="PSUM"))
        acc_pool = ctx.enter_context(tc.tile_pool(name="acc", bufs=2, space="PSUM"))
        attn_pool = ctx.enter_context(tc.tile_pool(name="attn", bufs=6))
        small_pool = ctx.enter_context(tc.tile_pool(name="small", bufs=4))
        dram_pool = ctx.enter_context(tc.tile_pool(name="dram", bufs=4, space="DRAM"))
        out_pool = ctx.enter_context(tc.tile_pool(name="outp", bufs=2))

        kT_sb = singles.tile([128, HKV // 2, SK], BF)
        nc.sync.dma_start(out=kT_sb, in_=kT_d[:, :, :])
        qT_sb = singles.tile([128, QT_N, SQ], BF)
        nc.sync.dma_start(out=qT_sb, in_=qT_d[:, :, :])
        vE_sb = singles.tile([128, HKV, KT, VE], BF)
        nc.sync.dma_start(out=vE_sb, in_=vE_d[:, :, :, :])
        wT_sb = singles.tile([128, PROJ_T, HID], BF)
        nc.sync.dma_start(out=wT_sb, in_=wT_d[:, :, :])

        oT_sb = singles.tile([128, PROJ_T, SQ], BF)

        # ---- attention: per (kv head, q-group) ----
        for kvh in range(HKV):
            kvpair, half = kvh // 2, kvh % 2
            for g in range(GRP):
                qp = kvpair * GRP + g
                h = kvh * GRP + g
                rhs_q = qT_sb[half * 64:(half + 1) * 64, qp, :]  # [64, SQ]
                pv = acc_pool.tile([128, SQ], FP32, tag="acc")
                for ktp in range(KT // 2):
                    qk = qk_pool.tile([128, 2 * SQ], FP32, tag="qk")
                    for j in (0, 1):
                        kt = 2 * ktp + j
                        lhsT_k = kT_sb[half * 64:(half + 1) * 64, kvpair,
                                       kt * 128:(kt + 1) * 128]  # [64, 128]
                        nc.tensor.matmul(
                            qk[:, j * SQ:(j + 1) * SQ], lhsT_k, rhs_q,
                            start=True, stop=True)
                    at = attn_pool.tile([128, 2 * SQ], BF, tag="at")
                    nc.scalar.activation(
                        out=at, in_=qk, func=mybir.ActivationFunctionType.Exp)
                    for j in (0, 1):
                        kt = 2 * ktp + j
                        nc.tensor.matmul(
                            pv[0:65, :], vE_sb[:, kvh, kt, 0:65],
                            at[:, j * SQ:(j + 1) * SQ],
                            start=(kt == 0), stop=(kt == KT - 1))
                # normalize: rows 0..63 by reciprocal of row 64 (softmax sums)
                rec = small_pool.tile([1, SQ], FP32, tag="rec")
                nc.vector.reciprocal(rec, pv[64:65, :])
                rec_dr = dram_pool.tile([1, SQ], FP32, tag="recd")
                nc.sync.dma_start(out=rec_dr, in_=rec)
                recb = small_pool.tile([64, SQ], FP32, tag="recb")
                bcast = bass.AP(tensor=rec_dr.tensor, offset=rec_dr.offset,
                                ap=[[0, 64], [1, SQ]])
                nc.sync.dma_start(out=recb, in_=bcast)
                o_un = small_pool.tile([64, SQ], FP32, tag="oun")
                nc.vector.tensor_copy(o_un, pv[0:64, :])
                t, hh = h // 2, h % 2
                nc.vector.tensor_mul(
                    oT_sb[hh * 64:(hh + 1) * 64, t, :], o_un, recb)

        # ---- out projection ----
        for qt in range(SQ // 128):
            out_sb = out_pool.tile([128, HID], FP32, tag="osb")
            for ht in range(HID_T):
                acc = acc_pool.tile([128, 512], FP32, tag="acc")
                for t in range(PROJ_T):
                    nc.tensor.matmul(
                        acc, oT_sb[:, t, qt * 128:(qt + 1) * 128],
                        wT_sb[:, t, ht * 512:(ht + 1) * 512],
                        start=(t == 0), stop=(t == PROJ_T - 1))
                nc.vector.tensor_copy(out_sb[:, ht * 512:(ht + 1) * 512], acc)
            nc.sync.dma_start(out=out_d[qt * 128:(qt + 1) * 128, :], in_=out_sb)

    nc.compile()
    return nc


def get_nc():
    global _cached
    if _cached is None:
        _cached = _build_program()
    return _cached


def prep_inputs(Q, K, V, W_out):
    """Host-side reshape/transpose/cast to the device layouts (per-core maps)."""
    Q = np.asarray(Q, np.float32)
    K = np.asarray(K, np.float32)
    V = np.asarray(V, np.float32)
    W_out = np.asarray(W_out, np.float32)

    # kT[b, p, pair, s] = K[b, s, (2*pair + p//64)*64 + p%64]
    kT = K.reshape(B, S, HKV, D).transpose(0, 2, 3, 1)        # [b,kvh,d,s]
    kT = kT.reshape(B, HKV // 2, 2, D, S).transpose(0, 2, 3, 1, 4)
    kT = np.ascontiguousarray(kT.reshape(B, 128, HKV // 2, S)).astype(BF16)

    # qT[b, qc, p, qp, j] = Q[b, qc*SQ+j, h*64+d]*SCALE, h=8*pair+4*half+g
    qT = Q.reshape(B, NC_PER_B, SQ, HQ, D).transpose(0, 1, 3, 4, 2)  # [b,qc,h,d,j]
    qT = qT.reshape(B, NC_PER_B, HKV // 2, 2, GRP, D, SQ)
    qT = qT.transpose(0, 1, 3, 5, 2, 4, 6)                    # [b,qc,half,d,pair,g,j]
    qT = (qT.reshape(B, NC_PER_B, 128, QT_N, SQ) * SCALE).astype(BF16)

    # vE[b, p, kvh, kt, e] = V[b, kt*128+p, kvh*64+e]; col 64 = ones
    vE = np.zeros((B, 128, HKV, KT, VE), np.float32)
    vE[..., :D] = V.reshape(B, KT, 128, HKV, D).transpose(0, 2, 3, 1, 4)
    vE[..., D] = 1.0
    vE = vE.astype(BF16)

    # wT[p, t, o] = W_out[o, t*128+p]
    wT = np.ascontiguousarray(
        W_out.T.reshape(PROJ_T, 128, HID).transpose(1, 0, 2)).astype(BF16)

    in_maps = []
    for c in range(8):
        b, qc = c // NC_PER_B, c % NC_PER_B
        in_maps.append({
            "qT": np.ascontiguousarray(qT[b, qc]),
            "kT": kT[b],
            "vE": vE[b],
            "wT": wT,
        })
    return in_maps


def run(inputs, trace=False, **kw):
    nc = get_nc()
    in_maps = prep_inputs(inputs["Q"], inputs["K"], inputs["V"], inputs["W_out"])
    res = run_bass_kernel_spmd(nc, in_maps, list(range(8)), trace=trace, **kw)
    out = np.empty((B, S, HID), np.float32)
    for c in range(8):
        b, qc = c // NC_PER_B, c % NC_PER_B
        out[b, qc * SQ:(qc + 1) * SQ, :] = res.results[c]["out"]
    out += np.asarray(inputs["b_out"], np.float32)
    return out, res


def kernel(**inputs):
    return run(inputs)[0]
